# revision 54
# baseline (speedup 1.0000x reference)
"""L2-distance self-attention (B=2, N=2048, D=1024, H=16) on 8 trn2 NeuronCores.

Sharding: core c handles batch c//4 and heads 4*(c%4) .. 4*(c%4)+4.
Each core computes its 4 heads end-to-end and returns TWO (2048, 1024) fp16
partials of the output projection (head pair 0 and head pair 1); the host
sums the 8 partials per batch and adds bo_eff = bo + wo @ bv (the v-bias
contributes exactly wo@bv after softmax normalization, so it is folded out
of the device kernel).

Layout: q_aug rows = [qb(0-63); ones(64); q2(65)], k_stat rows =
[kb2(0-63); k2(64); ones(65)] so one K=66 matmul emits the full
d2[j,i] = q2[i] + k2[j] - 2 q.k.  Row 65 of q_aug is written by an
SBUF->SBUF DMA (engines cannot address single partitions above 64, DMA
can).  kb2 = -2*(x wk + bk) is host-prescaled via wk/bk.

Per-head pipeline (ACT is the bottleneck at ~64us/head):
  sqrt phase: j-blocks 0-7 are consumed by ACT Sqrt straight from PSUM
    (two [128,1024] half-ops per block); blocks 8-15 are drained by DVE
    copies into the s tile and ACT runs Sqrt in-place over two 4-block
    groups.  This splits the drain work between ACT and DVE so neither
    paces the other.
  exp phase: ACT Exp (scale=-1) over strided i-chunks; PE runs PV
    matmuls (v_aug ones column -> row 64 = softmax denominator) plus the
    first j-block of the NEXT head's d2 so the next sqrt phase starts hot.
  normalize: denominator staged to SBUF fp32 at exp-phase end (frees the
    pv psum slot for the whole next sqrt phase; also reciprocal_approx_fast
    misreads PSUM on HW), then approx reciprocal -> PE broadcast matmul ->
    DVE multiply into oTp, per i-half in DVE-idle exp phases (head 3's
    tail half in i-quarters to shorten the tail).
  out-proj: pair-0 y during heads 2/3 sqrt phases, pair-1 y overlapping
    exp(3) and the tail; separate DRAM tensor per pair.
"""

import sys

for p in ("/opt/trn_rl_repo", "/root/.axon_site/_ro/trn_rl_repo"):
    if p not in sys.path:
        sys.path.append(p)

import numpy as np

B, N, D, H = 2, 2048, 1024, 16
HD = 64          # head dim
HPC = 4          # heads per core
HS = HPC * HD    # head-group width per core (256)
NB = N // 128    # 16 j-blocks
IC = N // 512    # 4 i-chunks of 512
KB = D // 128    # 8 contraction blocks for projections
EC = 256         # exp/PV i-chunk width
NEC = N // EC    # 8 exp chunks per head
NDIR = 8         # j-blocks consumed psum-direct by ACT (rest DVE-drained)

_CACHE = {}


def _build():
    import concourse.bacc as bacc
    import concourse.mybir as mybir
    import concourse.tile as tile

    dt = mybir.dt
    AF = mybir.ActivationFunctionType
    ALU = mybir.AluOpType

    nc = bacc.Bacc("TRN2", target_bir_lowering=False, debug=False)

    # ---- DRAM I/O (per core) ----
    xT = nc.dram_tensor("xT", [D, N], dt.float16, kind="ExternalInput")
    wq = nc.dram_tensor("wq_t", [D, HS], dt.float16, kind="ExternalInput")
    wk = nc.dram_tensor("wk_t", [D, HS], dt.float16, kind="ExternalInput")
    wv = nc.dram_tensor("wv_t", [D, HS], dt.float16, kind="ExternalInput")
    wo = nc.dram_tensor("woT", [HS, D], dt.float16, kind="ExternalInput")
    bias_d = nc.dram_tensor("biases", [128, 4], dt.float32, kind="ExternalInput")
    y0 = nc.dram_tensor("y0", [N, D], dt.float16, kind="ExternalOutput")
    y1 = nc.dram_tensor("y1", [N, D], dt.float16, kind="ExternalOutput")
    ydram = [y0, y1]

    with tile.TileContext(nc) as tc:
        with (
            tc.tile_pool(name="cst", bufs=1) as cst,
            tc.tile_pool(name="u4", bufs=9) as u4,        # 4KB slots: xt, sq, yac
            tc.tile_pool(name="wp", bufs=1) as wp,
            tc.tile_pool(name="wop", bufs=1) as wop,
            tc.tile_pool(name="aug", bufs=1) as aug,
            tc.tile_pool(name="rawp", bufs=2) as rawp,    # raws[h] rotate
            tc.tile_pool(name="dp", bufs=1) as dpool,
            tc.tile_pool(name="spool", bufs=1) as spool,
            tc.tile_pool(name="e8", bufs=2) as e8,
            tc.tile_pool(name="psum", bufs=2, space="PSUM") as ps,
        ):
            # ---- constants ----
            ones_row = cst.tile([1, 512], dt.float16, tag="ones_row")
            nc.gpsimd.memset(ones_row[:], 1.0)
            ones64f = cst.tile([1, 64], dt.float32, tag="ones64f")
            nc.gpsimd.memset(ones64f[:], 1.0)
            # norm reduce matrix: col0 = 1 on rows 0-63 (q2 = sum qb^2),
            # col32 = 0.25 on rows 64-127 (k2 = 0.25*sum kb2^2)
            emat = cst.tile([128, 33], dt.float16, tag="emat")
            nc.gpsimd.memset(emat[:], 0.0)
            nc.gpsimd.memset(emat[0:64, 0:1], 1.0)
            nc.gpsimd.memset(emat[64:128, 32:33], 0.25)

            bias_pp = cst.tile([128, 4], dt.float32, tag="bias_pp")
            nc.sync.dma_start(bias_pp[:], bias_d[:, :])
            # exp shift: e' = exp(5 - s); cancels in the softmax normalize
            bias5 = cst.tile([128, 1], dt.float32, tag="bias5")
            nc.gpsimd.memset(bias5[:], 5.0)

            # ---- per-head tiles ----
            q_aug = [aug.tile([66, N], dt.float16, tag=f"qa{h}", name=f"qa{h}") for h in range(HPC)]
            k_stat = [aug.tile([66, N], dt.float16, tag=f"ks{h}", name=f"ks{h}") for h in range(HPC)]
            for h in range(HPC):
                # q_aug row 64 = ones (const); row 65 overwritten with q2 by
                # DMA.  k_stat row 65 = ones (const); row 64 overwritten
                # with k2 by a DVE copy (base-64 is engine-addressable).
                nc.gpsimd.memset(q_aug[h][64:66, :], 1.0)
                nc.gpsimd.memset(k_stat[h][64:66, :], 1.0)
            # v per j-block: [p, h(4), d(65)]; d 64 = ones column -> PV row
            # 64 = softmax denominator
            v_jb = [aug.tile([128, HPC * 65], dt.float16, tag=f"v{jb}", name=f"v{jb}") for jb in range(NB)]
            for jb in range(NB):
                nc.gpsimd.memset(
                    v_jb[jb][:].rearrange("p (b d) -> p b d", d=65)[:, :, 64:65], 1.0
                )
            oTp = [
                aug.tile([128, N], dt.float16, tag="oTp0", name="oTp0"),
                aug.tile([128, N], dt.float16, tag="oTp1", name="oTp1"),
            ]

            # PE warmup: dependency-free matmuls release the HAM clock gate
            wup = ps.tile([128, 512], dt.float32, tag="pA", name="wup")
            for r in range(8):
                nc.tensor.matmul(
                    wup[:], ones_row[0:1, 0:128], ones_row[0:1, :],
                    start=(r == 0), stop=(r == 7),
                )

            # ---- input DMA ----
            xt = [u4.tile([128, N], dt.float16, tag="u4", name=f"xt{k}") for k in range(KB)]
            wq_all = wp.tile([128, KB * HS], dt.float16, tag="wq_all")
            wk_all = wp.tile([128, KB * HS], dt.float16, tag="wk_all")
            wv_all = wp.tile([128, KB * HS], dt.float16, tag="wv_all")
            for k in range(KB):
                nc.sync.dma_start(
                    wq_all[:, k * HS : (k + 1) * HS], wq[k * 128 : (k + 1) * 128, :]
                )
                nc.sync.dma_start(xt[k][:], xT[k * 128 : (k + 1) * 128, :])
            for k in range(KB):
                nc.sync.dma_start(
                    wk_all[:, k * HS : (k + 1) * HS], wk[k * 128 : (k + 1) * 128, :]
                )
            wotp = [wop.tile([128, D], dt.float16, tag=f"wop{p}", name=f"wop{p}") for p in range(2)]

            # ---- big SBUF tiles ----
            s = spool.tile([128, NB * N], dt.float16, tag="s")
            sv = s[:].rearrange("p (t i) -> p t i", t=NB)

            raws = [None] * HPC
            pend_st = {}   # (h, jb) -> (tileA, tileB) kept in PSUM
            pend_den = {}  # h -> staged B-half den row (sbuf fp32)
            pend_stA1 = {} # h -> pre-staged A-half psum of d2 block 1

            ic_sl = lambda ic: slice(ic * 512, (ic + 1) * 512)

            # ---- helpers (emission) ----
            def proj_qk_half(w_all, bcol, dest, m, half, tag):
                # heads 2m, 2m+1; psum (128 d, 1024 i); bias fused in copy
                lo = half * 1024
                p = ps.tile([128, 1024], dt.float32, tag=tag, name="pp")
                for d_ in range(2):
                    for k in range(KB):
                        nc.tensor.matmul(
                            p[:, d_ * 512 : (d_ + 1) * 512],
                            w_all[:, k * HS + m * 128 : k * HS + (m + 1) * 128],
                            xt[k][:, lo + d_ * 512 : lo + (d_ + 1) * 512],
                            start=(k == 0), stop=(k == KB - 1),
                        )
                for hf in range(2):
                    with nc.allow_low_precision(reason="fp16 activations"):
                        nc.vector.tensor_scalar_add(
                            out=dest[2 * m + hf][0:64, lo : lo + 1024],
                            in0=p[64 * hf : 64 * hf + 64, :],
                            scalar1=bias_pp[64 * hf : 64 * hf + 64, bcol + m : bcol + m + 1],
                        )

            def sq_half(h, which, sq, half):
                # squares for one i-half (1024 cols)
                lo = half * 1024
                src = q_aug[h] if which == "q" else k_stat[h]
                r0 = 0 if which == "q" else 64
                nc.vector.tensor_tensor(
                    out=sq[r0 : r0 + 64, lo : lo + 1024],
                    in0=src[0:64, lo : lo + 1024], in1=src[0:64, lo : lo + 1024],
                    op=ALU.mult,
                )

            def norm_red(h, which, sq, half, tag):
                # q2 (row 0) and k2 (row 32) of psum; q2 -> stage -> DMA to
                # q_aug row 65, k2 -> k_stat row 64 (direct DVE copy)
                lo = half * 1024
                p = ps.tile([33, 1024], dt.float32, tag=tag, name="np")
                for d_ in range(2):
                    ic = 2 * half + d_
                    nc.tensor.matmul(
                        p[:, d_ * 512 : (d_ + 1) * 512], emat[:], sq[:, ic_sl(ic)],
                        start=True, stop=True,
                    )
                with nc.allow_low_precision(reason="fp16 stats"):
                    if which == "q":
                        st_ = dpool.tile([1, 1024], dt.float16, tag="q2st", bufs=1, name="q2st")
                        nc.vector.tensor_copy(st_[:], p[0:1, :])
                        nc.gpsimd.dma_start(q_aug[h][65:66, lo : lo + 1024], st_[:])
                    else:
                        nc.vector.tensor_copy(k_stat[h][64:65, lo : lo + 1024], p[32:33, :])

            def vp_one(jb):
                p = ps.tile([128, HS], dt.float32, tag="pA" if jb % 2 == 0 else "pB", name="vp")
                for k in range(KB):
                    nc.tensor.matmul(
                        p[:], xt[k][:, jb * 128 : (jb + 1) * 128],
                        wv_all[:, k * HS : (k + 1) * HS],
                        start=(k == 0), stop=(k == KB - 1),
                    )
                dst = v_jb[jb][:].rearrange("p (h d) -> p h d", d=65)[:, :, 0:64]
                nc.vector.tensor_copy(dst, p[:].rearrange("p (h d) -> p h d", d=64))

            def st_halves(h, jb):
                # full d2 for j-block jb, both i-halves, kept in PSUM
                ts = []
                for half, tag in ((0, "pA"), (1, "pB")):
                    t_ = ps.tile([128, 1024], dt.float32, tag=tag, name=f"st{half}")
                    for d_ in range(2):
                        nc.tensor.matmul(
                            t_[:, d_ * 512 : (d_ + 1) * 512],
                            k_stat[h][0:66, jb * 128 : (jb + 1) * 128],
                            q_aug[h][0:66, half * 1024 + d_ * 512 : half * 1024 + (d_ + 1) * 512],
                            start=True, stop=True,
                        )
                    ts.append(t_)
                return ts

            def st_half_one(h, jb, half, tag):
                t_ = ps.tile([128, 1024], dt.float32, tag=tag, name=f"st{half}")
                for d_ in range(2):
                    nc.tensor.matmul(
                        t_[:, d_ * 512 : (d_ + 1) * 512],
                        k_stat[h][0:66, jb * 128 : (jb + 1) * 128],
                        q_aug[h][0:66, half * 1024 + d_ * 512 : half * 1024 + (d_ + 1) * 512],
                        start=True, stop=True,
                    )
                return t_

            def st_cast(h, jb, ts):
                # drain d2 psum -> s (fp16); sqrt later runs in-place
                for half in (0, 1):
                    with nc.allow_low_precision(reason="fp16 d2"):
                        nc.vector.tensor_copy(
                            s[:, jb * N + half * 1024 : jb * N + (half + 1) * 1024],
                            ts[half][:],
                        )

            def sqrt_psum_direct(h, jb, ts):
                for half in (0, 1):
                    nc.scalar.activation(
                        s[:, jb * N + half * 1024 : jb * N + (half + 1) * 1024],
                        ts[half][:], AF.Sqrt,
                    )

            def sqrt_group(g):
                lo, hi = 4 * g * N, 4 * (g + 1) * N
                nc.scalar.activation(s[:, lo:hi], s[:, lo:hi], AF.Sqrt)

            def exp_act(h, c):
                e = e8.tile([128, NB * EC], dt.float16, tag="e8", name="e")
                nc.scalar.activation(
                    e[:].rearrange("p (t i) -> p t i", t=NB),
                    sv[:, :, c * EC : (c + 1) * EC],
                    AF.Exp, scale=-1.0,
                )
                return e

            def pv_mms(h, c, pvh, e=None):
                cc = (c % 4) * EC
                for t in range(NB):
                    nc.tensor.matmul(
                        pvh[:, cc : cc + EC],
                        v_jb[t][:, h * 65 : h * 65 + 65],
                        e[:, t * EC : (t + 1) * EC],
                        start=(t == 0), stop=(t == NB - 1),
                    )

            def exp_chunk(h, c, pvh):
                e = exp_act(h, c)
                pv_mms(h, c, pvh, e)

            def raw_part(h, lo, w, pvh, plo):
                # pv rows 0-63 -> raws fp16 (row 64 = den stays in psum)
                if lo == 0:
                    raws[h] = rawp.tile([64, N], dt.float16, tag="raw", name=f"raw{h}")
                with nc.allow_low_precision(reason="fp16 softmax weights"):
                    nc.vector.tensor_copy(raws[h][:, lo : lo + w], pvh[0:64, plo : plo + w])

            def den_copy(h, w, pvh, plo):
                # den row psum -> sbuf; staging this at exp-phase end frees
                # the pv psum slot for the whole next sqrt phase
                den = dpool.tile([1, 1024], dt.float32, tag="den", bufs=2, name="den")
                nc.vector.tensor_copy(den[0:1, 0:w], pvh[64:65, plo : plo + w])
                return den

            def norm_part(h, lo, w, den, tag):
                # den -> approx reciprocal -> PE broadcast -> multiply raws
                # into oTp
                dinv = dpool.tile([1, 1024], dt.float32, tag="dinv", bufs=1, name="dinv")
                nc.vector.reciprocal_approx_fast(out=dinv[0:1, 0:w], in_=den[0:1, 0:w])
                bc = ps.tile([64, 1024], dt.float32, tag=tag, name="bc")
                for d_ in range((w + 511) // 512):
                    wd = min(512, w - d_ * 512)
                    nc.tensor.matmul(
                        bc[:, d_ * 512 : d_ * 512 + wd],
                        ones64f[:],
                        dinv[0:1, d_ * 512 : d_ * 512 + wd],
                        start=True, stop=True,
                    )
                row = 64 * (h % 2)
                with nc.allow_low_precision(reason="fp16 softmax weights"):
                    nc.vector.tensor_tensor(
                        out=oTp[h // 2][row : row + 64, lo : lo + w],
                        in0=raws[h][:, lo : lo + w], in1=bc[:, 0:w], op=ALU.mult,
                    )

            sqh = {}

            def norms_half(h, half):
                if h not in sqh:
                    sqh[h] = u4.tile([128, N], dt.float16, tag="u4", name=f"sqh{h}")
                sq = sqh[h]
                sq_half(h, "q", sq, half)
                sq_half(h, "k", sq, half)
                norm_red(h, "q", sq, half, "pA")
                norm_red(h, "k", sq, half, "pB")

            def yout(pair, ib, tag, act_copy=False):
                yp = ps.tile([128, D], dt.float32, tag=tag, name="yp")
                for fc in range(2):
                    nc.tensor.matmul(
                        yp[:, fc * 512 : (fc + 1) * 512],
                        oTp[pair][:, ib * 128 : (ib + 1) * 128],
                        wotp[pair][:, fc * 512 : (fc + 1) * 512],
                        start=True, stop=True,
                    )
                yac = u4.tile([128, D], dt.float16, tag="u4", name="yac")
                with nc.allow_low_precision(reason="fp16 partial output"):
                    if act_copy:
                        nc.scalar.copy(yac[:], yp[:])
                    else:
                        nc.vector.tensor_copy(yac[:], yp[:])
                nc.sync.dma_start(ydram[pair][ib * 128 : (ib + 1) * 128, :], yac[:])

            # ================= emission schedule =================
            # ---- lead-in: pair-0 q proj + q2; k proj i-half A; first d2 ----
            sqh[0] = u4.tile([128, N], dt.float16, tag="u4", name="sq0")
            # rows 64-127 are read (x0) by the q norm_reds before the k
            # squares land -- uninitialized NaN bits would poison the PE sum
            nc.gpsimd.memset(sqh[0][64:128, :], 0.0)
            # i-half A chain first so the first d2 block starts ASAP
            # (q before k: wq/xT lead the DMA queue, wk arrives last)
            proj_qk_half(wq_all, 0, q_aug, 0, 0, "pA")
            sq_half(0, "q", sqh[0], 0)
            proj_qk_half(wk_all, 2, k_stat, 0, 0, "pB")
            sq_half(0, "k", sqh[0], 0)
            norm_red(0, "q", sqh[0], 0, "pA")
            norm_red(0, "k", sqh[0], 0, "pB")
            proj_qk_half(wq_all, 0, q_aug, 0, 1, "pA")
            sq_half(0, "q", sqh[0], 1)
            norm_red(0, "q", sqh[0], 1, "pB")
            pend_st[(0, 0)] = st_halves(0, 0)
            # v / wo loads go behind the q2 DMAs on the SWDGE queue
            for k in range(KB):
                nc.gpsimd.dma_start(
                    wv_all[:, k * HS : (k + 1) * HS], wv[k * 128 : (k + 1) * 128, :]
                )
            for p_ in range(2):
                nc.gpsimd.dma_start(wotp[p_][:], wo[p_ * 128 : (p_ + 1) * 128, :])

            def k_stats_b():
                sq_half(0, "k", sqh[0], 1)
                norm_red(0, "k", sqh[0], 1, "pB")

            sqrt_fills = {
                0: [lambda: proj_qk_half(wk_all, 2, k_stat, 0, 1, "pA"), k_stats_b,
                    lambda: proj_qk_half(wq_all, 0, q_aug, 1, 0, "pB"),
                    lambda: proj_qk_half(wq_all, 0, q_aug, 1, 1, "pA"),
                    lambda: proj_qk_half(wk_all, 2, k_stat, 1, 0, "pB"),
                    lambda: proj_qk_half(wk_all, 2, k_stat, 1, 1, "pA")]
                   + [lambda jb=jb: vp_one(jb) for jb in range(NB)],
                1: [],
                2: [lambda ib=ib: yout(0, ib, "pB" if ib % 2 == 0 else "pA")
                    for ib in range(2, 6)],
                3: [lambda ib=ib: yout(0, ib, "pB" if ib % 2 == 0 else "pA")
                    for ib in range(12, NB)],
            }
            exp_fills = {
                (0, 2): [lambda: norms_half(1, 0)],
                (0, 5): [lambda: norms_half(1, 1)],
                (1, 1): [lambda: norms_half(2, 0)],
                (1, 2): [lambda: norms_half(2, 1)],
                (1, 4): [lambda: norms_half(3, 0)],
                (1, 5): [lambda: yout(0, 0, "pA")],
                (1, 6): [lambda: norms_half(3, 1)],
                (1, 7): [lambda: yout(0, 1, "pA")],
                (2, 1): [lambda: yout(0, 6, "pA")],
                (2, 2): [lambda: yout(0, 7, "pB")],
                (2, 4): [lambda: yout(0, 8, "pA")],
                (2, 5): [lambda: yout(0, 9, "pB")],
                (2, 6): [lambda: yout(0, 10, "pA")],
                (2, 7): [lambda: yout(0, 11, "pB")],
                (3, 4): [lambda: yout(1, 0, "pA"), lambda: yout(1, 1, "pA")],
                (3, 6): [lambda: yout(1, 2, "pA"), lambda: yout(1, 3, "pA"),
                         lambda: yout(1, 4, "pB"), lambda: yout(1, 5, "pA")],
                (3, 7): [lambda: yout(1, 6, "pB"), lambda: yout(1, 7, "pA")],
            }

            # ---- per-head phases ----
            for h in range(HPC):
                # ---------- sqrt phase ----------
                fills = iter(sqrt_fills[h])

                def fill(n=1):
                    for _ in range(n):
                        f = next(fills, None)
                        if f is not None:
                            f()

                # B-half normalize of previous head early in this phase
                ts0 = pend_st.pop((h, 0))
                sqrt_psum_direct(h, 0, ts0)
                if h > 0:
                    norm_part(h - 1, 1024, 1024, pend_den.pop(h - 1), "pA")
                # PE order interleaves drain-block d2 with direct-block d2 so
                # the DVE drains spread over the whole phase and the group
                # sqrts fire right after the psum-direct ones.  h==0 delays
                # the drains until the pair-0 k i-half-B stats fills land.
                dstart = 3 if h == 0 else 1
                dj_iter = iter(range(NDIR, NB))
                for jb in range(1, NDIR):
                    if jb == 1 and h in pend_stA1:
                        ts = [pend_stA1.pop(h), st_half_one(h, 1, 1, "pB")]
                    else:
                        ts = st_halves(h, jb)
                    sqrt_psum_direct(h, jb, ts)
                    fill(1)
                    if jb >= dstart:
                        dj = next(dj_iter, None)
                        if dj is not None:
                            st_cast(h, dj, st_halves(h, dj))
                for dj in dj_iter:
                    st_cast(h, dj, st_halves(h, dj))
                    fill(1)
                for g in range(NDIR // 4, 4):
                    sqrt_group(g)
                fill(100)

                # ---------- exp phase ----------
                pvA = ps.tile([65, 1024], dt.float32, tag="pA", name="pvA")
                pvB = ps.tile([65, 1024], dt.float32, tag="pB", name="pvB")
                for c in range(NEC):
                    if c == NEC - 1 and h < HPC - 1:
                        # next head's first d2 block lands inside the final
                        # exp window (ahead of PV(7) in the PE queue)
                        e_last = exp_act(h, c)
                        pend_st[(h + 1, 0)] = st_halves(h + 1, 0)
                        # A-half of block 1 also fits in the free pA slot
                        pend_stA1[h + 1] = st_half_one(h + 1, 1, 0, "pA")
                        pv_mms(h, c, pvB, e_last)
                    else:
                        exp_chunk(h, c, pvA if c < 4 else pvB)
                    if c == 3:
                        raw_part(h, 0, 1024, pvA, 0)
                        norm_part(h, 0, 1024, den_copy(h, 1024, pvA, 0), "pA")
                    if c == 5 and h == 3:
                        # head-3 B first quarter (cols 1024-1535) is final
                        raw_part(3, 1024, 512, pvB, 0)
                        norm_part(3, 1024, 512, den_copy(3, 512, pvB, 0), "pA")
                        for ib in range(8, 12):
                            yout(1, ib, "pA")
                    if c == 6 and h == 3:
                        # next eighth (cols 1536-1791) right behind chunk 6
                        raw_part(3, 1536, 256, pvB, 512)
                        norm_part(3, 1536, 256, den_copy(3, 256, pvB, 512), "pA")
                        yout(1, 12, "pA")
                        yout(1, 13, "pA")
                    for f in exp_fills.get((h, c), []):
                        f()
                if h < HPC - 1:
                    raw_part(h, 1024, 1024, pvB, 0)
                    pend_den[h] = den_copy(h, 1024, pvB, 0)

            # ---------- tail: head-3 B second quarter, rest of pair-1 y ----
            raw_part(3, 1792, 256, pvB, 768)
            norm_part(3, 1792, 256, den_copy(3, 256, pvB, 768), "pB")
            for i_, ib in enumerate(range(14, NB)):
                yout(1, ib, "pB" if ib % 2 == 0 else "pA", act_copy=(i_ % 2 == 0))

    nc.compile()
    return nc


def _prep_in_maps(x, wq, bq, wk, bk, wv, wo):
    f16 = np.float16
    in_maps = []
    xTs = [np.ascontiguousarray(x[b].T).astype(f16) for b in range(B)]
    for c in range(8):
        b, hg = divmod(c, HPC)
        hs = hg * HS
        biases = np.stack(
            [
                bq[hs : hs + 128],
                bq[hs + 128 : hs + 256],
                -2.0 * bk[hs : hs + 128],
                -2.0 * bk[hs + 128 : hs + 256],
            ],
            axis=1,
        ).astype(np.float32)
        in_maps.append(
            {
                "xT": xTs[b],
                "wq_t": np.ascontiguousarray(wq[hs : hs + HS, :].T).astype(f16),
                "wk_t": np.ascontiguousarray(-2.0 * wk[hs : hs + HS, :].T).astype(f16),
                "wv_t": np.ascontiguousarray(wv[hs : hs + HS, :].T).astype(f16),
                "woT": np.ascontiguousarray(wo[:, hs : hs + HS].T).astype(f16),
                "biases": np.ascontiguousarray(biases),
            }
        )
    return in_maps


def _get_nc():
    if "nc" not in _CACHE:
        _CACHE["nc"] = _build()
    return _CACHE["nc"]


def run(inputs, trace=False, **trace_kwargs):
    """Run on 8 cores; returns (full_output, BassKernelResults)."""
    from concourse.bass_utils import run_bass_kernel_spmd

    nc = _get_nc()
    wv_np = np.asarray(inputs["wv"], np.float32)
    bv_np = np.asarray(inputs["bv"], np.float32)
    wo_np = np.asarray(inputs["wo"], np.float32)
    in_maps = _prep_in_maps(
        np.asarray(inputs["x"], np.float32),
        np.asarray(inputs["wq"], np.float32), np.asarray(inputs["bq"], np.float32),
        np.asarray(inputs["wk"], np.float32), np.asarray(inputs["bk"], np.float32),
        wv_np, wo_np,
    )
    res = run_bass_kernel_spmd(nc, in_maps, list(range(8)), trace=trace, **trace_kwargs)
    # v-bias folds to wo @ bv after softmax normalization
    bo_eff = np.asarray(inputs["bo"], np.float32) + wo_np @ bv_np
    out = np.empty((B, N, D), np.float32)
    for b in range(B):
        acc = res.results[b * HPC]["y0"].astype(np.float32)
        acc = acc + res.results[b * HPC]["y1"].astype(np.float32)
        for c in range(b * HPC + 1, (b + 1) * HPC):
            acc = acc + res.results[c]["y0"].astype(np.float32)
            acc = acc + res.results[c]["y1"].astype(np.float32)
        out[b] = acc + bo_eff
    return out, res


def kernel(**inputs) -> np.ndarray:
    out, _ = run(inputs, trace=False)
    return out


if __name__ == "__main__":
    rng = np.random.default_rng(0)
    ins = {
        "x": rng.standard_normal((B, N, D)).astype(np.float32),
        "wq": (rng.standard_normal((D, D)) * 0.02).astype(np.float32),
        "bq": (rng.standard_normal(D) * 0.02).astype(np.float32),
        "wk": (rng.standard_normal((D, D)) * 0.02).astype(np.float32),
        "bk": (rng.standard_normal(D) * 0.02).astype(np.float32),
        "wv": (rng.standard_normal((D, D)) * 0.02).astype(np.float32),
        "bv": (rng.standard_normal(D) * 0.02).astype(np.float32),
        "wo": (rng.standard_normal((D, D)) * 0.02).astype(np.float32),
        "bo": (rng.standard_normal(D) * 0.02).astype(np.float32),
    }
    print(kernel(**ins).shape)


# revision 56
# speedup vs baseline: 1.0300x; 1.0300x over previous
"""L2-distance self-attention (B=2, N=2048, D=1024, H=16) on 8 trn2 NeuronCores.

Sharding: core c handles batch c//4 and heads 4*(c%4) .. 4*(c%4)+4.
Each core computes its 4 heads end-to-end and returns TWO (2048, 1024) fp16
partials of the output projection (head pair 0 and head pair 1); the host
sums the 8 partials per batch and adds bo_eff = bo + wo @ bv (the v-bias
contributes exactly wo@bv after softmax normalization, so it is folded out
of the device kernel).

Layout: q_aug rows = [qb(0-63); ones(64); q2(65)], k_stat rows =
[kb2(0-63); k2(64); ones(65)] so one K=66 matmul emits the full
d2[j,i] = q2[i] + k2[j] - 2 q.k.  Row 65 of q_aug is written by an
SBUF->SBUF DMA (engines cannot address single partitions above 64, DMA
can).  kb2 = -2*(x wk + bk) is host-prescaled via wk/bk.

Per-head pipeline (ACT is the bottleneck at ~64us/head):
  sqrt phase: j-blocks 0-7 are consumed by ACT Sqrt straight from PSUM
    (two [128,1024] half-ops per block); blocks 8-15 are drained by DVE
    copies into the s tile and ACT runs Sqrt in-place over two 4-block
    groups.  This splits the drain work between ACT and DVE so neither
    paces the other.
  exp phase: ACT Exp (scale=-1) over strided i-chunks; PE runs PV
    matmuls (v_aug ones column -> row 64 = softmax denominator) plus the
    first j-block of the NEXT head's d2 so the next sqrt phase starts hot.
  normalize: denominator staged to SBUF fp32 at exp-phase end (frees the
    pv psum slot for the whole next sqrt phase; also reciprocal_approx_fast
    misreads PSUM on HW), then approx reciprocal -> PE broadcast matmul ->
    DVE multiply into oTp, per i-half in DVE-idle exp phases (head 3's
    tail half in i-quarters to shorten the tail).
  out-proj: pair-0 y during heads 2/3 sqrt phases, pair-1 y overlapping
    exp(3) and the tail; separate DRAM tensor per pair.
"""

import sys

for p in ("/opt/trn_rl_repo", "/root/.axon_site/_ro/trn_rl_repo"):
    if p not in sys.path:
        sys.path.append(p)

import numpy as np

B, N, D, H = 2, 2048, 1024, 16
HD = 64          # head dim
HPC = 4          # heads per core
HS = HPC * HD    # head-group width per core (256)
NB = N // 128    # 16 j-blocks
IC = N // 512    # 4 i-chunks of 512
KB = D // 128    # 8 contraction blocks for projections
EC = 256         # exp/PV i-chunk width
NEC = N // EC    # 8 exp chunks per head
NDIR = 8         # j-blocks consumed psum-direct by ACT (rest DVE-drained)

_CACHE = {}


def _build():
    import concourse.bacc as bacc
    import concourse.mybir as mybir
    import concourse.tile as tile

    dt = mybir.dt
    AF = mybir.ActivationFunctionType
    ALU = mybir.AluOpType

    nc = bacc.Bacc("TRN2", target_bir_lowering=False, debug=False)

    # ---- DRAM I/O (per core) ----
    xT = nc.dram_tensor("xT", [D, N], dt.float16, kind="ExternalInput")
    wq = nc.dram_tensor("wq_t", [D, HS], dt.float16, kind="ExternalInput")
    wk = nc.dram_tensor("wk_t", [D, HS], dt.float16, kind="ExternalInput")
    wv = nc.dram_tensor("wv_t", [D, HS], dt.float16, kind="ExternalInput")
    wo = nc.dram_tensor("woT", [HS, D], dt.float16, kind="ExternalInput")
    bias_d = nc.dram_tensor("biases", [128, 4], dt.float32, kind="ExternalInput")
    y0 = nc.dram_tensor("y0", [N, D], dt.float16, kind="ExternalOutput")
    y1 = nc.dram_tensor("y1", [N, D], dt.float16, kind="ExternalOutput")
    ydram = [y0, y1]

    with tile.TileContext(nc) as tc:
        with (
            tc.tile_pool(name="cst", bufs=1) as cst,
            tc.tile_pool(name="u4", bufs=9) as u4,        # 4KB slots: xt, sq, yac
            tc.tile_pool(name="wp", bufs=1) as wp,
            tc.tile_pool(name="wop", bufs=1) as wop,
            tc.tile_pool(name="aug", bufs=1) as aug,
            tc.tile_pool(name="rawp", bufs=2) as rawp,    # raws[h] rotate
            tc.tile_pool(name="dp", bufs=1) as dpool,
            tc.tile_pool(name="spool", bufs=1) as spool,
            tc.tile_pool(name="e8", bufs=2) as e8,
            tc.tile_pool(name="psum", bufs=2, space="PSUM") as ps,
        ):
            # ---- constants ----
            ones_row = cst.tile([1, 512], dt.float16, tag="ones_row")
            nc.gpsimd.memset(ones_row[:], 1.0)
            ones64f = cst.tile([1, 64], dt.float32, tag="ones64f")
            nc.gpsimd.memset(ones64f[:], 1.0)
            # norm reduce matrix: col0 = 1 on rows 0-63 (q2 = sum qb^2),
            # col32 = 0.25 on rows 64-127 (k2 = 0.25*sum kb2^2)
            emat = cst.tile([128, 33], dt.float16, tag="emat")
            nc.gpsimd.memset(emat[:], 0.0)
            nc.gpsimd.memset(emat[0:64, 0:1], 1.0)
            nc.gpsimd.memset(emat[64:128, 32:33], 0.25)

            bias_pp = cst.tile([128, 4], dt.float32, tag="bias_pp")
            nc.sync.dma_start(bias_pp[:], bias_d[:, :])
            # exp shift: e' = exp(5 - s); cancels in the softmax normalize
            bias5 = cst.tile([128, 1], dt.float32, tag="bias5")
            nc.gpsimd.memset(bias5[:], 5.0)

            # ---- per-head tiles ----
            q_aug = [aug.tile([66, N], dt.float16, tag=f"qa{h}", name=f"qa{h}") for h in range(HPC)]
            k_stat = [aug.tile([66, N], dt.float16, tag=f"ks{h}", name=f"ks{h}") for h in range(HPC)]
            for h in range(HPC):
                # q_aug row 64 = ones (const); row 65 overwritten with q2 by
                # DMA.  k_stat row 65 = ones (const); row 64 overwritten
                # with k2 by a DVE copy (base-64 is engine-addressable).
                nc.gpsimd.memset(q_aug[h][64:66, :], 1.0)
                nc.gpsimd.memset(k_stat[h][64:66, :], 1.0)
            # v per j-block: [p, h(4), d(65)]; d 64 = ones column -> PV row
            # 64 = softmax denominator
            v_jb = [aug.tile([128, HPC * 65], dt.float16, tag=f"v{jb}", name=f"v{jb}") for jb in range(NB)]
            for jb in range(NB):
                nc.gpsimd.memset(
                    v_jb[jb][:].rearrange("p (b d) -> p b d", d=65)[:, :, 64:65], 1.0
                )
            oTp = [
                aug.tile([128, N], dt.float16, tag="oTp0", name="oTp0"),
                aug.tile([128, N], dt.float16, tag="oTp1", name="oTp1"),
            ]

            # PE warmup: dependency-free matmuls release the HAM clock gate
            wup = ps.tile([128, 512], dt.float32, tag="pA", name="wup")
            for r in range(8):
                nc.tensor.matmul(
                    wup[:], ones_row[0:1, 0:128], ones_row[0:1, :],
                    start=(r == 0), stop=(r == 7),
                )

            # ---- input DMA ----
            xt = [u4.tile([128, N], dt.float16, tag="u4", name=f"xt{k}") for k in range(KB)]
            wq_all = wp.tile([128, KB * HS], dt.float16, tag="wq_all")
            wk_all = wp.tile([128, KB * HS], dt.float16, tag="wk_all")
            wv_all = wp.tile([128, KB * HS], dt.float16, tag="wv_all")
            for k in range(KB):
                nc.sync.dma_start(
                    wq_all[:, k * HS : (k + 1) * HS], wq[k * 128 : (k + 1) * 128, :]
                )
                nc.sync.dma_start(xt[k][:], xT[k * 128 : (k + 1) * 128, :])
            for k in range(KB):
                nc.sync.dma_start(
                    wk_all[:, k * HS : (k + 1) * HS], wk[k * 128 : (k + 1) * 128, :]
                )
            wotp = [wop.tile([128, D], dt.float16, tag=f"wop{p}", name=f"wop{p}") for p in range(2)]

            # ---- big SBUF tiles ----
            s = spool.tile([128, NB * N], dt.float16, tag="s")
            sv = s[:].rearrange("p (t i) -> p t i", t=NB)

            raws = [None] * HPC
            pend_st = {}   # (h, jb) -> (tileA, tileB) kept in PSUM
            pend_den = {}  # h -> staged B-half den row (sbuf fp32)
            pend_stA1 = {} # h -> pre-staged A-half psum of d2 block 1

            ic_sl = lambda ic: slice(ic * 512, (ic + 1) * 512)

            # ---- helpers (emission) ----
            def proj_qk_half(w_all, bcol, dest, m, half, tag):
                # heads 2m, 2m+1; psum (128 d, 1024 i); bias fused in copy
                lo = half * 1024
                p = ps.tile([128, 1024], dt.float32, tag=tag, name="pp")
                for d_ in range(2):
                    for k in range(KB):
                        nc.tensor.matmul(
                            p[:, d_ * 512 : (d_ + 1) * 512],
                            w_all[:, k * HS + m * 128 : k * HS + (m + 1) * 128],
                            xt[k][:, lo + d_ * 512 : lo + (d_ + 1) * 512],
                            start=(k == 0), stop=(k == KB - 1),
                        )
                for hf in range(2):
                    with nc.allow_low_precision(reason="fp16 activations"):
                        nc.vector.tensor_scalar_add(
                            out=dest[2 * m + hf][0:64, lo : lo + 1024],
                            in0=p[64 * hf : 64 * hf + 64, :],
                            scalar1=bias_pp[64 * hf : 64 * hf + 64, bcol + m : bcol + m + 1],
                        )

            def sq_half(h, which, sq, half):
                # squares for one i-half (1024 cols)
                lo = half * 1024
                src = q_aug[h] if which == "q" else k_stat[h]
                r0 = 0 if which == "q" else 64
                nc.vector.tensor_tensor(
                    out=sq[r0 : r0 + 64, lo : lo + 1024],
                    in0=src[0:64, lo : lo + 1024], in1=src[0:64, lo : lo + 1024],
                    op=ALU.mult,
                )

            def norm_red(h, which, sq, half, tag):
                # q2 (row 0) and k2 (row 32) of psum; q2 -> stage -> DMA to
                # q_aug row 65, k2 -> k_stat row 64 (direct DVE copy)
                lo = half * 1024
                p = ps.tile([33, 1024], dt.float32, tag=tag, name="np")
                for d_ in range(2):
                    ic = 2 * half + d_
                    nc.tensor.matmul(
                        p[:, d_ * 512 : (d_ + 1) * 512], emat[:], sq[:, ic_sl(ic)],
                        start=True, stop=True,
                    )
                with nc.allow_low_precision(reason="fp16 stats"):
                    if which == "q":
                        st_ = dpool.tile([1, 1024], dt.float16, tag="q2st", bufs=1, name="q2st")
                        nc.vector.tensor_copy(st_[:], p[0:1, :])
                        nc.gpsimd.dma_start(q_aug[h][65:66, lo : lo + 1024], st_[:])
                    else:
                        nc.vector.tensor_copy(k_stat[h][64:65, lo : lo + 1024], p[32:33, :])

            def vp_one(jb):
                p = ps.tile([128, HS], dt.float32, tag="pA" if jb % 2 == 0 else "pB", name="vp")
                for k in range(KB):
                    nc.tensor.matmul(
                        p[:], xt[k][:, jb * 128 : (jb + 1) * 128],
                        wv_all[:, k * HS : (k + 1) * HS],
                        start=(k == 0), stop=(k == KB - 1),
                    )
                dst = v_jb[jb][:].rearrange("p (h d) -> p h d", d=65)[:, :, 0:64]
                nc.vector.tensor_copy(dst, p[:].rearrange("p (h d) -> p h d", d=64))

            def st_halves(h, jb):
                # full d2 for j-block jb, both i-halves, kept in PSUM
                ts = []
                for half, tag in ((0, "pA"), (1, "pB")):
                    t_ = ps.tile([128, 1024], dt.float32, tag=tag, name=f"st{half}")
                    for d_ in range(2):
                        nc.tensor.matmul(
                            t_[:, d_ * 512 : (d_ + 1) * 512],
                            k_stat[h][0:66, jb * 128 : (jb + 1) * 128],
                            q_aug[h][0:66, half * 1024 + d_ * 512 : half * 1024 + (d_ + 1) * 512],
                            start=True, stop=True,
                        )
                    ts.append(t_)
                return ts

            def st_half_one(h, jb, half, tag):
                t_ = ps.tile([128, 1024], dt.float32, tag=tag, name=f"st{half}")
                for d_ in range(2):
                    nc.tensor.matmul(
                        t_[:, d_ * 512 : (d_ + 1) * 512],
                        k_stat[h][0:66, jb * 128 : (jb + 1) * 128],
                        q_aug[h][0:66, half * 1024 + d_ * 512 : half * 1024 + (d_ + 1) * 512],
                        start=True, stop=True,
                    )
                return t_

            def st_cast(h, jb, ts):
                # drain d2 psum -> s (fp16); sqrt later runs in-place
                for half in (0, 1):
                    with nc.allow_low_precision(reason="fp16 d2"):
                        nc.vector.tensor_copy(
                            s[:, jb * N + half * 1024 : jb * N + (half + 1) * 1024],
                            ts[half][:],
                        )

            def sqrt_psum_direct(h, jb, ts):
                for half in (0, 1):
                    nc.scalar.activation(
                        s[:, jb * N + half * 1024 : jb * N + (half + 1) * 1024],
                        ts[half][:], AF.Sqrt,
                    )

            def sqrt_group(g):
                lo, hi = 4 * g * N, 4 * (g + 1) * N
                nc.scalar.activation(s[:, lo:hi], s[:, lo:hi], AF.Sqrt)

            def exp_act(h, c):
                e = e8.tile([128, NB * EC], dt.float16, tag="e8", name="e")
                nc.scalar.activation(
                    e[:].rearrange("p (t i) -> p t i", t=NB),
                    sv[:, :, c * EC : (c + 1) * EC],
                    AF.Exp, scale=-1.0,
                )
                return e

            def pv_mms(h, c, pvh, e=None):
                cc = (c % 4) * EC
                for t in range(NB):
                    nc.tensor.matmul(
                        pvh[:, cc : cc + EC],
                        v_jb[t][:, h * 65 : h * 65 + 65],
                        e[:, t * EC : (t + 1) * EC],
                        start=(t == 0), stop=(t == NB - 1),
                    )

            def exp_chunk(h, c, pvh):
                e = exp_act(h, c)
                pv_mms(h, c, pvh, e)

            def raw_part(h, lo, w, pvh, plo):
                # pv rows 0-63 -> raws fp16 (row 64 = den stays in psum)
                if lo == 0:
                    raws[h] = rawp.tile([64, N], dt.float16, tag="raw", name=f"raw{h}")
                with nc.allow_low_precision(reason="fp16 softmax weights"):
                    nc.vector.tensor_copy(raws[h][:, lo : lo + w], pvh[0:64, plo : plo + w])

            def den_copy(h, w, pvh, plo):
                # den row psum -> sbuf; staging this at exp-phase end frees
                # the pv psum slot for the whole next sqrt phase
                den = dpool.tile([1, 1024], dt.float32, tag="den", bufs=2, name="den")
                nc.vector.tensor_copy(den[0:1, 0:w], pvh[64:65, plo : plo + w])
                return den

            def norm_part(h, lo, w, den, tag):
                # den -> approx reciprocal -> PE broadcast -> multiply raws
                # into oTp
                dinv = dpool.tile([1, 1024], dt.float32, tag="dinv", bufs=1, name="dinv")
                nc.vector.reciprocal_approx_fast(out=dinv[0:1, 0:w], in_=den[0:1, 0:w])
                bc = ps.tile([64, 1024], dt.float32, tag=tag, name="bc")
                for d_ in range((w + 511) // 512):
                    wd = min(512, w - d_ * 512)
                    nc.tensor.matmul(
                        bc[:, d_ * 512 : d_ * 512 + wd],
                        ones64f[:],
                        dinv[0:1, d_ * 512 : d_ * 512 + wd],
                        start=True, stop=True,
                    )
                row = 64 * (h % 2)
                with nc.allow_low_precision(reason="fp16 softmax weights"):
                    nc.vector.tensor_tensor(
                        out=oTp[h // 2][row : row + 64, lo : lo + w],
                        in0=raws[h][:, lo : lo + w], in1=bc[:, 0:w], op=ALU.mult,
                    )

            sqh = {}

            def norms_half(h, half):
                if h not in sqh:
                    sqh[h] = u4.tile([128, N], dt.float16, tag="u4", name=f"sqh{h}")
                sq = sqh[h]
                sq_half(h, "q", sq, half)
                sq_half(h, "k", sq, half)
                norm_red(h, "q", sq, half, "pA")
                norm_red(h, "k", sq, half, "pB")

            def yout(pair, ib, tag, act_copy=False):
                yp = ps.tile([128, D], dt.float32, tag=tag, name="yp")
                for fc in range(2):
                    nc.tensor.matmul(
                        yp[:, fc * 512 : (fc + 1) * 512],
                        oTp[pair][:, ib * 128 : (ib + 1) * 128],
                        wotp[pair][:, fc * 512 : (fc + 1) * 512],
                        start=True, stop=True,
                    )
                yac = u4.tile([128, D], dt.float16, tag="u4", name="yac")
                with nc.allow_low_precision(reason="fp16 partial output"):
                    if act_copy:
                        nc.scalar.copy(yac[:], yp[:])
                    else:
                        nc.vector.tensor_copy(yac[:], yp[:])
                nc.sync.dma_start(ydram[pair][ib * 128 : (ib + 1) * 128, :], yac[:])

            # ================= emission schedule =================
            # ---- lead-in: pair-0 q proj + q2; k proj i-half A; first d2 ----
            sqh[0] = u4.tile([128, N], dt.float16, tag="u4", name="sq0")
            # rows 64-127 are read (x0) by the q norm_reds before the k
            # squares land -- uninitialized NaN bits would poison the PE sum
            nc.gpsimd.memset(sqh[0][64:128, :], 0.0)
            # i-half A chain first so the first d2 block starts ASAP
            # (q before k: wq/xT lead the DMA queue, wk arrives last)
            proj_qk_half(wq_all, 0, q_aug, 0, 0, "pA")
            sq_half(0, "q", sqh[0], 0)
            proj_qk_half(wk_all, 2, k_stat, 0, 0, "pB")
            sq_half(0, "k", sqh[0], 0)
            norm_red(0, "q", sqh[0], 0, "pA")
            norm_red(0, "k", sqh[0], 0, "pB")
            proj_qk_half(wq_all, 0, q_aug, 0, 1, "pA")
            sq_half(0, "q", sqh[0], 1)
            norm_red(0, "q", sqh[0], 1, "pB")
            pend_st[(0, 0)] = st_halves(0, 0)
            # v / wo loads go behind the q2 DMAs on the SWDGE queue
            for k in range(KB):
                nc.gpsimd.dma_start(
                    wv_all[:, k * HS : (k + 1) * HS], wv[k * 128 : (k + 1) * 128, :]
                )
            for p_ in range(2):
                nc.gpsimd.dma_start(wotp[p_][:], wo[p_ * 128 : (p_ + 1) * 128, :])

            def k_stats_b():
                sq_half(0, "k", sqh[0], 1)
                norm_red(0, "k", sqh[0], 1, "pB")

            sqrt_fills = {
                0: [lambda: proj_qk_half(wk_all, 2, k_stat, 0, 1, "pA"), k_stats_b,
                    lambda: proj_qk_half(wq_all, 0, q_aug, 1, 0, "pB"),
                    lambda: proj_qk_half(wq_all, 0, q_aug, 1, 1, "pA"),
                    lambda: proj_qk_half(wk_all, 2, k_stat, 1, 0, "pB"),
                    lambda: proj_qk_half(wk_all, 2, k_stat, 1, 1, "pA")]
                   + [lambda jb=jb: vp_one(jb) for jb in range(NB)],
                1: [],
                2: [lambda ib=ib: yout(0, ib, "pB" if ib % 2 == 0 else "pA",
                                        act_copy=True)
                    for ib in range(2, 6)],
                3: [lambda ib=ib: yout(0, ib, "pB" if ib % 2 == 0 else "pA",
                                        act_copy=True)
                    for ib in range(12, NB)],
            }
            exp_fills = {
                (0, 2): [lambda: norms_half(1, 0)],
                (0, 5): [lambda: norms_half(1, 1)],
                (1, 1): [lambda: norms_half(2, 0)],
                (1, 2): [lambda: norms_half(2, 1)],
                (1, 4): [lambda: norms_half(3, 0)],
                (1, 5): [lambda: yout(0, 0, "pA")],
                (1, 6): [lambda: norms_half(3, 1)],
                (1, 7): [lambda: yout(0, 1, "pA")],
                (2, 1): [lambda: yout(0, 6, "pA")],
                (2, 2): [lambda: yout(0, 7, "pB")],
                (2, 4): [lambda: yout(0, 8, "pA")],
                (2, 5): [lambda: yout(0, 9, "pB")],
                (2, 6): [lambda: yout(0, 10, "pA")],
                (2, 7): [lambda: yout(0, 11, "pB")],
                (3, 4): [lambda: yout(1, 0, "pA"), lambda: yout(1, 1, "pA")],
                (3, 6): [lambda: yout(1, 2, "pA"), lambda: yout(1, 3, "pA"),
                         lambda: yout(1, 4, "pB"), lambda: yout(1, 5, "pA")],
                (3, 7): [lambda: yout(1, 6, "pB"), lambda: yout(1, 7, "pA")],
            }

            # ---- per-head phases ----
            for h in range(HPC):
                # ---------- sqrt phase ----------
                fills = iter(sqrt_fills[h])

                def fill(n=1):
                    for _ in range(n):
                        f = next(fills, None)
                        if f is not None:
                            f()

                # B-half normalize of previous head early in this phase
                ts0 = pend_st.pop((h, 0))
                sqrt_psum_direct(h, 0, ts0)
                if h > 0:
                    norm_part(h - 1, 1024, 1024, pend_den.pop(h - 1), "pA")
                # PE order interleaves drain-block d2 with direct-block d2 so
                # the DVE drains spread over the whole phase and the group
                # sqrts fire right after the psum-direct ones.  h==0 delays
                # the drains until the pair-0 k i-half-B stats fills land.
                dstart = 3 if h == 0 else 1
                dj_iter = iter(range(NDIR, NB))
                for jb in range(1, NDIR):
                    if jb == 1 and h in pend_stA1:
                        ts = [pend_stA1.pop(h), st_half_one(h, 1, 1, "pB")]
                    else:
                        ts = st_halves(h, jb)
                    sqrt_psum_direct(h, jb, ts)
                    fill(1)
                    if jb >= dstart:
                        dj = next(dj_iter, None)
                        if dj is not None:
                            st_cast(h, dj, st_halves(h, dj))
                for dj in dj_iter:
                    st_cast(h, dj, st_halves(h, dj))
                    fill(1)
                for g in range(NDIR // 4, 4):
                    sqrt_group(g)
                fill(100)

                # ---------- exp phase ----------
                pvA = ps.tile([65, 1024], dt.float32, tag="pA", name="pvA")
                pvB = ps.tile([65, 1024], dt.float32, tag="pB", name="pvB")
                for c in range(NEC):
                    if c == NEC - 1 and h < HPC - 1:
                        # next head's first d2 block lands inside the final
                        # exp window (ahead of PV(7) in the PE queue)
                        e_last = exp_act(h, c)
                        pend_st[(h + 1, 0)] = st_halves(h + 1, 0)
                        # A-half of block 1 also fits in the free pA slot
                        pend_stA1[h + 1] = st_half_one(h + 1, 1, 0, "pA")
                        pv_mms(h, c, pvB, e_last)
                    else:
                        exp_chunk(h, c, pvA if c < 4 else pvB)
                    if c == 3:
                        raw_part(h, 0, 1024, pvA, 0)
                        norm_part(h, 0, 1024, den_copy(h, 1024, pvA, 0), "pA")
                    if c == 5 and h == 3:
                        # head-3 B first quarter (cols 1024-1535) is final
                        raw_part(3, 1024, 512, pvB, 0)
                        norm_part(3, 1024, 512, den_copy(3, 512, pvB, 0), "pA")
                        for ib in range(8, 12):
                            yout(1, ib, "pA")
                    for f in exp_fills.get((h, c), []):
                        f()
                if h < HPC - 1:
                    raw_part(h, 1024, 1024, pvB, 0)
                    pend_den[h] = den_copy(h, 1024, pvB, 0)

            # ---------- tail: head-3 B second quarter, rest of pair-1 y ----
            raw_part(3, 1536, 512, pvB, 512)
            norm_part(3, 1536, 512, den_copy(3, 512, pvB, 512), "pB")
            for i_, ib in enumerate(range(12, NB)):
                yout(1, ib, "pB" if ib % 2 == 0 else "pA", act_copy=(i_ % 2 == 0))

    nc.compile()
    return nc


def _prep_in_maps(x, wq, bq, wk, bk, wv, wo):
    f16 = np.float16
    in_maps = []
    xTs = [np.ascontiguousarray(x[b].T).astype(f16) for b in range(B)]
    for c in range(8):
        b, hg = divmod(c, HPC)
        hs = hg * HS
        biases = np.stack(
            [
                bq[hs : hs + 128],
                bq[hs + 128 : hs + 256],
                -2.0 * bk[hs : hs + 128],
                -2.0 * bk[hs + 128 : hs + 256],
            ],
            axis=1,
        ).astype(np.float32)
        in_maps.append(
            {
                "xT": xTs[b],
                "wq_t": np.ascontiguousarray(wq[hs : hs + HS, :].T).astype(f16),
                "wk_t": np.ascontiguousarray(-2.0 * wk[hs : hs + HS, :].T).astype(f16),
                "wv_t": np.ascontiguousarray(wv[hs : hs + HS, :].T).astype(f16),
                "woT": np.ascontiguousarray(wo[:, hs : hs + HS].T).astype(f16),
                "biases": np.ascontiguousarray(biases),
            }
        )
    return in_maps


def _get_nc():
    if "nc" not in _CACHE:
        _CACHE["nc"] = _build()
    return _CACHE["nc"]


def run(inputs, trace=False, **trace_kwargs):
    """Run on 8 cores; returns (full_output, BassKernelResults)."""
    from concourse.bass_utils import run_bass_kernel_spmd

    nc = _get_nc()
    wv_np = np.asarray(inputs["wv"], np.float32)
    bv_np = np.asarray(inputs["bv"], np.float32)
    wo_np = np.asarray(inputs["wo"], np.float32)
    in_maps = _prep_in_maps(
        np.asarray(inputs["x"], np.float32),
        np.asarray(inputs["wq"], np.float32), np.asarray(inputs["bq"], np.float32),
        np.asarray(inputs["wk"], np.float32), np.asarray(inputs["bk"], np.float32),
        wv_np, wo_np,
    )
    res = run_bass_kernel_spmd(nc, in_maps, list(range(8)), trace=trace, **trace_kwargs)
    # v-bias folds to wo @ bv after softmax normalization
    bo_eff = np.asarray(inputs["bo"], np.float32) + wo_np @ bv_np
    out = np.empty((B, N, D), np.float32)
    for b in range(B):
        acc = res.results[b * HPC]["y0"].astype(np.float32)
        acc = acc + res.results[b * HPC]["y1"].astype(np.float32)
        for c in range(b * HPC + 1, (b + 1) * HPC):
            acc = acc + res.results[c]["y0"].astype(np.float32)
            acc = acc + res.results[c]["y1"].astype(np.float32)
        out[b] = acc + bo_eff
    return out, res


def kernel(**inputs) -> np.ndarray:
    out, _ = run(inputs, trace=False)
    return out


if __name__ == "__main__":
    rng = np.random.default_rng(0)
    ins = {
        "x": rng.standard_normal((B, N, D)).astype(np.float32),
        "wq": (rng.standard_normal((D, D)) * 0.02).astype(np.float32),
        "bq": (rng.standard_normal(D) * 0.02).astype(np.float32),
        "wk": (rng.standard_normal((D, D)) * 0.02).astype(np.float32),
        "bk": (rng.standard_normal(D) * 0.02).astype(np.float32),
        "wv": (rng.standard_normal((D, D)) * 0.02).astype(np.float32),
        "bv": (rng.standard_normal(D) * 0.02).astype(np.float32),
        "wo": (rng.standard_normal((D, D)) * 0.02).astype(np.float32),
        "bo": (rng.standard_normal(D) * 0.02).astype(np.float32),
    }
    print(kernel(**ins).shape)
